# revision 37
# baseline (speedup 1.0000x reference)
"""Trainium2 Bass kernel for LocalCrossCorrelationWithSmoothnessLoss.

Full inputs in, full output out. Pure data-parallel over the batch dim
(B=8 -> 8 NeuronCores); each core computes partial sums for its image;
the host combines them into the three scalar losses.

Per-core pipeline (one 1024x1024 image pair + two flow channels):
  load       I, J, s loaded as bf16 via SWDGE cast-DMA (gpsimd) --
             spreads across all 16 SDMA engines and needs no DVE casts.
  products   IJ (DVE TT bf16 2x), I^2 (ACT Square), J^2 (gpsimd), bf16.
  stage 1    H-direction 9-tap box conv as banded matmuls on the PE
             (band stationary bf16, map moving bf16); both 512-col
             halves land in one 2-bank f32 psum tile -> single FD=1024
             copy to a bf16 S-map chunk.  Product maps use an 81-scaled
             band so the combine is pure tensor work.
  transpose  REGULAR matmuls S.T @ ident (not transpose-mode: engages
             FWL and the HAM clock boost); pieces of map PAIRS are
             interleaved across two psum tiles so fill/drain overlap;
             pieces crossing the 512-col psum bank boundary split.
  stage 2    W-direction box conv on the transposed maps.  si/sj are
             squared (ACT, direct from psum) and copied to SBUF; the
             single-use maps sij/sii/sjj are consumed directly from
             psum by the combine TTs.
  combine    bf16, FD=1024: P = si*sj (gpsimd), crossN = sij - P,
             IvarN = sii - sqI, JvarN = sjj - sqJ (DVE TT from psum),
             denom (gpsimd), recip = exp(-ln(denom+eps)) (ACT),
             cc = crossN^2 * recip accumulated via DVE STT accum_out.
  smooth     dx: gpsimd shifted subtract + Square accum (ACT or DVE).
             dy: difference-band matmul on PE (psum = s[h+1]-s[h]) +
             ACT Square accum from PSUM.  Tile-boundary dy rows are
             host-corrected.  No SBUF->SBUF shift DMAs (those serialize
             onto a single SDMA engine).  Smoothness tiles interleave
             with stage-1 chunks and stage-2 chunks to fill troughs.

Output per core: 57 partial sums. Host assembles the losses in float64.
"""
import sys
import numpy as np

sys.path.insert(0, "/opt/trn_rl_repo")

import ml_dtypes
import bass_rust
import concourse.bass as bass
import concourse.tile as tile
from concourse import mybir
from concourse import bass_utils
from concourse import tile_utils

F32 = mybir.dt.float32
BF16 = mybir.dt.bfloat16
ALU = mybir.AluOpType
ACTF = mybir.ActivationFunctionType

H = 1024
W = 1024
PAD = 4
WIN = 81.0
ALPHA = 0.01
EPS = 1e-9
EPS_N = EPS * WIN * WIN    # eps in the 81x-scaled domain
STRIDE = 120

# chunk table: (out_lo, out_n, in_lo, in_n)
CHUNKS = []
for _c in range((H + STRIDE - 1) // STRIDE):
    _olo = STRIDE * _c
    _on = min(STRIDE, H - _olo)
    _ilo = max(0, _olo - PAD)
    _ihi = min(H, _olo + _on + PAD)
    CHUNKS.append((_olo, _on, _ilo, _ihi - _ilo))
NCH = len(CHUNKS)

# accumulator column layout
COL_CC = 0            # 9 cols, one per w-chunk
COL_DX = COL_CC + NCH          # 16 cols, one per (ch, tile)
COL_DY = COL_DX + 16           # 32 cols, one per (ch, tile, half)
NACC = COL_DY + 32             # 57

# allow using the full usable SBUF (tile_utils default is stale at 192K)
tile_utils.max_sbuf_usage = 206 * 1024


_nc_cache = {}


def _legalize_waits(nc, max_waits=1):
    """walrus here accepts only one sync-wait command per instruction;
    split extras onto same-engine NoOps placed just before."""
    ctr = 0
    for f in nc.m.functions:
        for bb in f.blocks:
            insts = bb.instructions
            i = 0
            while i < len(insts):
                ins = insts[i]
                si = ins.sync_info
                if si is None:
                    i += 1
                    continue
                w = list(si.on_wait)
                if len(w) <= max_waits:
                    i += 1
                    continue
                extra, keep = w[:-max_waits], w[-max_waits:]
                nops = []
                for j in range(0, len(extra), max_waits):
                    chunk = extra[j:j + max_waits]
                    nop = mybir.InstNoOp(name=f"I-wsplit-{ctr}", ins=[], outs=[])
                    ctr += 1
                    nop.engine = ins.engine
                    nop.sync_info = bass_rust.SyncInfo(on_wait=chunk, on_update=[])
                    nops.append(nop)
                ins.sync_info = bass_rust.SyncInfo(on_wait=keep,
                                                  on_update=list(si.on_update))
                insts[i:i] = nops
                i += len(nops) + 1


def _make_host_consts():
    """Band matrices (bf16), identity (bf16), diff band (bf16), ones."""
    def band(klo, kn, olo, on, scale):
        k = np.arange(klo, klo + kn)[:, None]
        m = np.arange(olo, olo + on)[None, :]
        return (np.abs(k - m) <= PAD).astype(np.float32) * scale

    bands = np.zeros((128, 4 * STRIDE), dtype=np.float32)
    # variant 0: first chunk (c=0), scale 1;  variant 1: first chunk, 81
    # variant 2: interior (c>=1), scale 1;    variant 3: interior, 81
    olo0, on0, ilo0, in0 = CHUNKS[0]
    bands[:in0, 0:on0] = band(ilo0, in0, olo0, on0, 1.0)
    bands[:in0, STRIDE:STRIDE + on0] = band(ilo0, in0, olo0, on0, 81.0)
    olo1, on1, ilo1, in1 = CHUNKS[1]
    bands[:in1, 2 * STRIDE:2 * STRIDE + on1] = band(ilo1, in1, olo1, on1, 1.0)
    bands[:in1, 3 * STRIDE:3 * STRIDE + on1] = band(ilo1, in1, olo1, on1, 81.0)
    bands_bf = bands.astype(ml_dtypes.bfloat16)
    ident_bf = np.eye(128, dtype=np.float32).astype(ml_dtypes.bfloat16)
    # difference band: out[m] = s[m+1] - s[m], m in [0, 126]
    dband = np.zeros((128, 128), dtype=np.float32)
    for m in range(127):
        dband[m + 1, m] = 1.0
        dband[m, m] = -1.0
    dband_bf = dband.astype(ml_dtypes.bfloat16)
    ones_f32 = np.ones((128, 1), dtype=np.float32)
    return bands_bf, ident_bf, dband_bf, ones_f32


def _band_ap(bands_t, c, scaled):
    """AP into the packed bands tile for chunk c."""
    olo, on, ilo, inn = CHUNKS[c]
    if c == 0:
        v = 1 if scaled else 0
    else:
        v = 3 if scaled else 2
    return bands_t[0:inn, v * STRIDE:v * STRIDE + on]


def _build(nc):
    I_d = nc.dram_tensor("I", [H, W], F32, kind="ExternalInput").ap()
    J_d = nc.dram_tensor("J", [H, W], F32, kind="ExternalInput").ap()
    s0_d = nc.dram_tensor("s0", [H, W], F32, kind="ExternalInput").ap()
    s1_d = nc.dram_tensor("s1", [H, W], F32, kind="ExternalInput").ap()
    bands_d = nc.dram_tensor("bands", [128, 4 * STRIDE], BF16,
                             kind="ExternalInput").ap()
    ident_d = nc.dram_tensor("ident", [128, 128], BF16,
                             kind="ExternalInput").ap()
    dband_d = nc.dram_tensor("dband", [128, 128], BF16,
                             kind="ExternalInput").ap()
    MAPS = ("si", "sj", "sij", "sii", "sjj")
    ones_d = nc.dram_tensor("ones", [128, 1], F32, kind="ExternalInput").ap()
    part_d = nc.dram_tensor("partials", [1, NACC], F32,
                            kind="ExternalOutput").ap()

    from contextlib import ExitStack
    with tile.TileContext(nc) as tc, ExitStack() as ctx:
        consts = ctx.enter_context(tc.tile_pool(name="consts", bufs=1))
        inp = ctx.enter_context(tc.tile_pool(name="inp", bufs=3))
        prod = ctx.enter_context(tc.tile_pool(name="prod", bufs=2))
        smap = ctx.enter_context(tc.tile_pool(name="smap", bufs=1))
        tmap = ctx.enter_context(tc.tile_pool(name="tmap", bufs=2))
        comb = ctx.enter_context(tc.tile_pool(name="comb", bufs=2))
        spool = ctx.enter_context(tc.tile_pool(name="spool", bufs=2))
        accp = ctx.enter_context(tc.tile_pool(name="accp", bufs=1))
        psA = ctx.enter_context(tc.tile_pool(name="psA", bufs=2, space="PSUM"))
        pbig = ctx.enter_context(tc.tile_pool(name="pbig", bufs=3,
                                              space="PSUM"))

        bands_t = consts.tile([128, 4 * STRIDE], BF16)
        ident_t = consts.tile([128, 128], BF16)
        dband_t = consts.tile([128, 128], BF16)
        ones_t = consts.tile([128, 1], F32)
        nc.sync.dma_start(bands_t[:], bands_d)
        nc.sync.dma_start(ident_t[:], ident_d)
        nc.sync.dma_start(dband_t[:], dband_d)
        nc.sync.dma_start(ones_t[:], ones_d)

        # accumulators: accum_out OVERWRITES, so every accumulating
        # instruction gets its own column; host sums the groups.
        acc = accp.tile([128, NACC], F32)
        nc.vector.memset(acc[:], 0.0)
        epsb = consts.tile([128, 1], F32)
        nc.vector.memset(epsb[:], EPS_N)

        # ---------------- smoothness tile (interleaved) -----------------
        def smooth_tile(k):
            ch_i, t = k // 8, k % 8
            s_d = s0_d if ch_i == 0 else s1_d
            st = spool.tile([128, W], BF16, tag="s_in")
            nc.gpsimd.dma_start(st[:], s_d[128 * t:128 * (t + 1), :])
            # dx: shifted subtract (gpsimd) + square-accum (DVE/ACT alt)
            sub = spool.tile([128, W], BF16, tag="s_sub")
            nc.gpsimd.tensor_tensor(out=sub[:, 0:W - 1], in0=st[:, 1:W],
                                    in1=st[:, 0:W - 1], op=ALU.subtract)
            junk = spool.tile([128, W], BF16, tag="s_junk")
            cx = COL_DX + ch_i * 8 + t
            if k % 2 == 0:
                nc.scalar.activation(junk[:, 0:W - 1], sub[:, 0:W - 1],
                                     ACTF.Square,
                                     accum_out=acc[:, cx:cx + 1])
            else:
                nc.vector.scalar_tensor_tensor(
                    out=junk[:, 0:W - 1], in0=sub[:, 0:W - 1], scalar=1.0,
                    in1=sub[:, 0:W - 1], op0=ALU.mult, op1=ALU.mult,
                    accum_out=acc[:, cx:cx + 1])
            # dy: difference-band matmul (PE) then Square accum from PSUM
            for hw in range(2):
                hsl = slice(512 * hw, 512 * hw + 512)
                pD = psA.tile([128, 512], F32, tag="psA",
                              padded_shape=[128, 512])
                nc.tensor.matmul(pD[0:127, :], dband_t[0:128, 0:127],
                                 st[0:128, hsl], start=True, stop=True)
                col = COL_DY + (ch_i * 8 + t) * 2 + hw
                nc.scalar.activation(junk[0:127, hsl], pD[0:127, :],
                                     ACTF.Square,
                                     accum_out=acc[0:127, col:col + 1])

        # ---------------- stage 1: H-conv -> S maps --------------------
        s_tiles = {}
        for c, (olo, on, ilo, inn) in enumerate(CHUNKS):
            I_t = inp.tile([128, W], BF16, tag="I_in")
            J_t = inp.tile([128, W], BF16, tag="J_in")
            nc.gpsimd.dma_start(I_t[0:inn, :], I_d[ilo:ilo + inn, :])
            nc.gpsimd.dma_start(J_t[0:inn, :], J_d[ilo:ilo + inn, :])

            IJ_t = prod.tile([128, W], BF16, tag="IJ")
            I2_t = prod.tile([128, W], BF16, tag="I2")
            J2_t = prod.tile([128, W], BF16, tag="J2")
            nc.vector.tensor_tensor(out=IJ_t[0:inn, :], in0=I_t[0:inn, :],
                                    in1=J_t[0:inn, :], op=ALU.mult)
            nc.scalar.square(I2_t[0:inn, :], I_t[0:inn, :])
            nc.gpsimd.tensor_tensor(out=J2_t[0:inn, :], in0=J_t[0:inn, :],
                                    in1=J_t[0:inn, :], op=ALU.mult)

            srcs = (I_t, J_t, IJ_t, I2_t, J2_t)
            for mi, name in enumerate(MAPS):
                scaled = mi >= 2
                p1 = pbig.tile([128, W], F32, tag="pb", name=f"p1_{name}")
                for hw in range(2):
                    wsl = slice(512 * hw, 512 * hw + 512)
                    nc.tensor.matmul(p1[0:on, wsl],
                                     _band_ap(bands_t, c, scaled),
                                     srcs[mi][0:inn, wsl],
                                     start=True, stop=True)
                ssb = smap.tile([128, W], BF16, tag=f"S_{name}_{c}",
                                name=f"S_{name}_{c}")
                s_tiles[(name, c)] = ssb
                if (c * 5 + mi) % 2 == 0:
                    nc.vector.tensor_copy(ssb[0:on, :], p1[0:on, :])
                else:
                    nc.scalar.copy(ssb[0:on, :], p1[0:on, :])

            # smoothness tiles 0..8 fill stage-1 troughs
            smooth_tile(c)

        # ------------- stage 2 per chunk: transpose, W-conv, combine ----
        for c2, (olo2, on2, ilo2, in2) in enumerate(CHUNKS):
            n = on2
            t_tiles = {}
            for mi, name in enumerate(MAPS):
                # transpose via REGULAR matmuls (S.T @ ident): engages the
                # HAM clock boost and FWL, unlike transpose-mode.  Output
                # is f32 psum; pieces crossing the 512-col bank boundary
                # split so each matmul stays within one bank.
                pT = pbig.tile([128, H], F32, tag="pb", name="pT")
                for chk, (holo, hon, _, _) in enumerate(CHUNKS):
                    stc = s_tiles[(name, chk)]
                    segs = ([(holo, 512), (512, holo + hon)]
                            if holo < 512 < holo + hon
                            else [(holo, holo + hon)])
                    for lo, hi in segs:
                        nc.tensor.matmul(
                            pT[0:in2, lo:hi],
                            stc[0:hon, ilo2:ilo2 + in2],
                            ident_t[0:hon, lo - holo:hi - holo],
                            start=True, stop=True,
                        )
                tt = tmap.tile([128, H], BF16, tag=f"T_{name}")
                if (c2 * 5 + mi) % 2 == 0:
                    nc.vector.tensor_copy(tt[0:in2, :], pT[0:in2, :])
                else:
                    nc.scalar.copy(tt[0:in2, :], pT[0:in2, :])
                t_tiles[name] = tt

            def s2_matmul(name):
                p2 = pbig.tile([128, H], F32, tag="pb", name=f"p2_{name}")
                for hw in range(2):
                    hsl = slice(512 * hw, 512 * hw + 512)
                    nc.tensor.matmul(p2[0:n, hsl],
                                     _band_ap(bands_t, c2, False),
                                     t_tiles[name][0:in2, hsl],
                                     start=True, stop=True)
                return p2

            # si / sj: square from psum (ACT) + copy to SBUF, psum frees
            si = comb.tile([128, H], BF16, tag="si")
            sj = comb.tile([128, H], BF16, tag="sj")
            sqI = comb.tile([128, H], BF16, tag="sqI")
            sqJ = comb.tile([128, H], BF16, tag="sqJ")
            p2si = s2_matmul("si")
            nc.scalar.square(sqI[0:n, :], p2si[0:n, :])
            nc.vector.tensor_copy(si[0:n, :], p2si[0:n, :])
            p2sj = s2_matmul("sj")
            nc.scalar.square(sqJ[0:n, :], p2sj[0:n, :])
            nc.scalar.copy(sj[0:n, :], p2sj[0:n, :])
            # P = si*sj on gpsimd (SBUF bf16)
            P = comb.tile([128, H], BF16, tag="P")
            nc.gpsimd.tensor_tensor(out=P[0:n, :], in0=si[0:n, :],
                                    in1=sj[0:n, :], op=ALU.mult)
            # single-use maps: combine reads psum directly (DVE TT 1x)
            crossN = comb.tile([128, H], BF16, tag="crossN")
            p2sij = s2_matmul("sij")
            nc.vector.tensor_tensor(out=crossN[0:n, :], in0=p2sij[0:n, :],
                                    in1=P[0:n, :], op=ALU.subtract)
            p2sii = s2_matmul("sii")
            nc.vector.tensor_tensor(out=sqI[0:n, :], in0=p2sii[0:n, :],
                                    in1=sqI[0:n, :], op=ALU.subtract)
            p2sjj = s2_matmul("sjj")
            nc.vector.tensor_tensor(out=sqJ[0:n, :], in0=p2sjj[0:n, :],
                                    in1=sqJ[0:n, :], op=ALU.subtract)
            # denom over sqI (gpsimd); recip = exp(-ln(denom+eps)) in-place
            nc.gpsimd.tensor_tensor(out=sqI[0:n, :], in0=sqI[0:n, :],
                                    in1=sqJ[0:n, :], op=ALU.mult)
            nc.scalar.activation(sqI[0:n, :], sqI[0:n, :], ACTF.Ln,
                                 bias=epsb[0:n, 0:1])
            nc.scalar.activation(sqI[0:n, :], sqI[0:n, :], ACTF.Exp,
                                 scale=-1.0)
            # crossN^2 in-place, then accumulate cc
            nc.scalar.square(crossN[0:n, :], crossN[0:n, :])
            nc.vector.scalar_tensor_tensor(
                out=P[0:n, :], in0=crossN[0:n, :], scalar=1.0,
                in1=sqI[0:n, :], op0=ALU.mult, op1=ALU.mult,
                accum_out=acc[0:n, COL_CC + c2:COL_CC + c2 + 1])

            # smoothness tiles 9..15 fill stage-2 troughs
            if c2 < 7:
                smooth_tile(9 + c2)

        # ---------------- final partition reduction ---------------------
        pF = psA.tile([128, 512], F32, tag="psA", name="pF")
        nc.tensor.matmul(pF[0:1, 0:NACC], ones_t[:], acc[:],
                         start=True, stop=True)
        outt = accp.tile([1, NACC], F32, tag="outt")
        nc.scalar.copy(outt[:], pF[0:1, 0:NACC])
        nc.sync.dma_start(part_d, outt[:])

    return
